# revision 55
# baseline (speedup 1.0000x reference)
"""Trainium2 Bass kernel: masked multi-head attention (B=2, S=2048, D=512, H=8).

Sharding: batch x head-pair across 8 cores (core = b*4 + head_pair).
Each core computes, for its batch b and its 2 heads:
    q/k/v projections -> scores^T -> exp (mask folded in as per-partition
    bias on the ScalarE) -> attn@v with a ones-column appended to V (gives
    the softmax denominator for free) -> normalize -> partial out-proj.
The 4 per-batch partials are summed on the host (the "all-reduce"), then
bias bo is added.

Design notes (from ntff/HAM profiling):
  * All matmuls stay in plain 128x128 PE mode (the v1 kernel mixed 64x128
    row-tiled scores with 128x128 attn@v -> a PE drain per mode switch and
    a HAM clock-gate stuck at 1.2 GHz).  Scores use zero-padded per-head
    kT tiles so lhsT/rhs always span all 128 partitions.
  * The pipeline is paced by the ScalarE exp stream (~1.03us per
    [128,1024] chunk-head).  PE order interleaves scores(c+lag)/attn@v(c);
    projections, the previous block's normalize and out-proj fill the gaps.
  * q/k staging is FP8 (e4m3, weights pre-scaled by 16, absorbed in the
    exp scale): the softmax temperature makes q/k quantization error
    negligible (~0.3%) while halving the critical staging bytes.  v stays
    bf16 (its error passes straight to the output).
  * All inputs are marshalled partition-major on the host so each DMA
    moves one large contiguous run per partition (DMA here is
    packet-rate-bound: 772B packets measured ~66GB/s, 4KB+ packets
    ~330GB/s).  One fused DMA per tensor (each dispatch costs ~650ns of
    sequencer time).
  * Softmax denominators: ones-column of v_aug -> PSUM row 64 ->
    reciprocal via a PE broadcast matmul (sel) + DVE, no DRAM bounce.
  * PSUM budget (8 banks): scores ring 2x[128,1024] (4 banks) + attn@v
    accumulators 2x[128,1024] (4 banks).  The previous tp's out-proj and
    the softmax-denominator broadcasts borrow freed slots of those rings.
"""

import math
import os
import sys

import numpy as np

for _p in ("/opt/trn_rl_repo",):
    if os.path.isdir(_p) and _p not in sys.path:
        sys.path.insert(0, _p)

import ml_dtypes

D_MODEL = 512
NUM_HEADS = 8
HEAD_DIM = 64
N_CORES = 8
LOCAL_F = 128          # features per core = 2 heads * 64
VAUG = 2 * HEAD_DIM + 2  # 130: [v_h0 (64) | ones | v_h1 (64) | ones]
MASK_NEG = -30000.0
WSCALE = 16.0          # fp8 pre-scale on Wq/Wk (absorbed in the exp scale)

DT_NAME = os.environ.get("ATTN_KERNEL_DT", "bfloat16")
TRACE = False

last_results = None  # BassKernelResults of the most recent run (for test.py)

_PROG_CACHE = {}


def _np_dt(name):
    return ml_dtypes.bfloat16 if name == "bfloat16" else np.float32


def _build(nch: int, seq: int, dt_name: str, qk_bias: bool, v_bias: bool):
    from contextlib import ExitStack

    import concourse.bass as bass  # noqa: F401
    import concourse.mybir as mybir
    import concourse.tile as tile
    from concourse import bacc

    DT = getattr(mybir.dt, dt_name)
    F32 = mybir.dt.float32
    F8 = mybir.dt.float8e4
    EXP = mybir.ActivationFunctionType.Exp
    sk = nch * 128
    n_tp = seq // 1024
    hs = seq // 2
    assert seq % 1024 == 0

    nc = bacc.Bacc("TRN2", target_bir_lowering=False, debug=False,
                   num_devices=N_CORES, num_swdge_queues=4)

    def din(name, shape, dt=DT):
        return nc.dram_tensor(name, shape, dt, kind="ExternalInput").ap()

    # partition-major inputs: [128, c, f] contiguous per partition;
    # q in 512-column blocks so the first projection starts ASAP
    nqb = seq // 512
    xqb_d = [din(f"xq{i}", [128, 4, 512], F8) for i in range(nqb)]
    xk_d = din("xk", [128, 4, sk], F8)
    xv_d = din("xv", [128, 4, sk])
    wqk_d = din("wqk", [128, 4, 2 * LOCAL_F], F8)
    wv_d = din("wv", [128, 4, VAUG])
    wo_d = din("wo", [LOCAL_F, D_MODEL])
    NSM = 2 + VAUG + nch
    smalls_d = din("smalls", [128, NSM], F32)
    sel_d = din("sel", [128, 128])
    out_d = nc.dram_tensor("out", [D_MODEL, seq], DT,
                           kind="ExternalOutput").ap()
    # DRAM bounce for the boundary softmax-denominator broadcast (runs on
    # otherwise-idle DMA queues; SBUF APs can't partition-broadcast)
    dnb_d = nc.dram_tensor("dnb", [max(n_tp - 1, 1), 2, 1024], F32).ap()

    with tile.TileContext(nc) as tc, ExitStack() as ctx:
        const = ctx.enter_context(tc.tile_pool(name="const", bufs=1))

        # ---- staging: critical-path tensors first on each queue ----
        xk_sb = const.tile([128, 4, sk], F8, tag="xk")
        nc.sync.dma_start(out=xk_sb, in_=xk_d)
        xq_sb = const.tile([128, 4, seq], F8, tag="xq")
        for i in range(nqb):
            nc.sync.dma_start(out=xq_sb[:, :, i * 512:(i + 1) * 512],
                              in_=xqb_d[i])

        wqk_sb = const.tile([128, 4, 2 * LOCAL_F], F8, tag="wqk")
        nc.scalar.dma_start(out=wqk_sb, in_=wqk_d)
        sm_sb = const.tile([128, NSM], F32, tag="sm")
        nc.scalar.dma_start(out=sm_sb, in_=smalls_d)
        wv_sb = const.tile([128, 4, VAUG], DT, tag="wv")
        nc.scalar.dma_start(out=wv_sb, in_=wv_d)
        sel = const.tile([128, 128], DT, tag="sel")
        nc.scalar.dma_start(out=sel, in_=sel_d)
        wo_sb = const.tile([LOCAL_F, D_MODEL], DT, tag="wo")
        nc.scalar.dma_start(out=wo_sb, in_=wo_d)
        # xv last: v-proj must not become ready before the first scores,
        # or the list scheduler runs it first and delays the exp stream
        xv_sb = const.tile([128, 4, sk], DT, tag="xv")
        nc.scalar.dma_start(out=xv_sb, in_=xv_d)

        bq_sb = sm_sb[:, 0:1]
        bk_sb = sm_sb[:, 1:2]
        bv_sb = sm_sb[:, 2:2 + VAUG]
        mb_sb = sm_sb[:, 2 + VAUG:2 + VAUG + nch]

        # ---- device-built constants (all memsets on VectorE) ----
        warm = const.tile([128, 512], DT, tag="warm")
        nc.vector.memset(warm, 0.0)
        # per-head zero-padded kT: keeps the scores matmul in 128x128 mode
        kTp = [const.tile([128, sk], DT, tag=f"kTp{h}", name=f"kTp{h}")
               for h in range(2)]
        nc.vector.memset(kTp[0][64:128, :], 0.0)
        nc.vector.memset(kTp[1][0:64, :], 0.0)
        # denominator staging rows 0/32 (h1 lands on partition 32: engine
        # APs need 32-aligned partition starts), rest must be zero
        dn = [const.tile([128, 1024], DT, tag=f"dn{i}", name=f"dn{i}")
              for i in range(2)]
        nc.vector.memset(dn[0], 0.0)
        nc.vector.memset(dn[1], 0.0)
        rb = [const.tile([128, 512], F32, tag=f"rb{i}", name=f"rb{i}")
              for i in range(2)]
        dnf = const.tile([128, 1024], F32, tag="dnf")
        rbf = const.tile([128, 1024], F32, tag="rbf")
        # SBUF copies of the attn@v numerators at tp boundaries: free the
        # psum accumulators early so the next tp's attn@v never waits on
        # the (DMA-latency-bound) normalize chain
        oS = const.tile([128, 1024], DT, tag="oS")

        qT = const.tile([LOCAL_F, seq], DT, tag="qT")
        vaug = const.tile([128, nch, VAUG], DT, tag="vaug")
        nc.vector.memset(vaug[:, :, 64:65], 1.0)
        nc.vector.memset(vaug[:, :, 129:130], 1.0)
        cn = const.tile([LOCAL_F, seq], DT, tag="cn")

        expp = ctx.enter_context(tc.tile_pool(name="expp", bufs=10))
        pool_sc = ctx.enter_context(
            tc.tile_pool(name="ps_sc", bufs=2, space="PSUM"))

        exs = {}

        def scores(tp, c, h):
            sc = pool_sc.tile([128, 1024], F32, tag="sc",
                              name=f"sc{tp}_{c}_{h}")
            q0 = tp * 1024
            for t in range(2):
                nc.tensor.matmul(
                    sc[:, t * 512:(t + 1) * 512],
                    lhsT=kTp[h][:, c * 128:(c + 1) * 128],
                    rhs=qT[:, q0 + t * 512:q0 + (t + 1) * 512],
                    start=True, stop=True,
                )
            ex = expp.tile([128, 1024], DT, tag="ex", name=f"ex{tp}_{c}_{h}")
            nc.scalar.activation(
                out=ex, in_=sc, func=EXP,
                bias=mb_sb[:, c:c + 1],
                scale=1.0 / (math.sqrt(HEAD_DIM) * WSCALE * WSCALE),
            )
            exs[(tp, c, h)] = ex

        def attnv(tp, c, h, oT, t=None):
            for tt in ((0, 1) if t is None else (t,)):
                nc.tensor.matmul(
                    oT[h][0:65, tt * 512:(tt + 1) * 512],
                    lhsT=vaug[:, c, h * 65:(h + 1) * 65],
                    rhs=exs[(tp, c, h)][:, tt * 512:(tt + 1) * 512],
                    start=(c == 0), stop=(c == nch - 1),
                )

        # ================= prelude: warm-up + projections ================
        PRE = min(2, nch)
        with tc.tile_pool(name="ps_p", bufs=2, space="PSUM") as ppj:
            wps = ppj.tile([128, 512], F32, tag="pwarm", name="warm_ps")
            for _ in range(4):
                nc.tensor.matmul(wps, lhsT=warm[:, 0:128], rhs=warm,
                                 start=True, stop=True)

            def proj_k(j0, w):
                ps = ppj.tile([128, 512], F32, tag="pj", name=f"pk{j0}")
                for dc in range(4):
                    nc.tensor.matmul(
                        ps[:, :w],
                        lhsT=wqk_sb[:, dc, LOCAL_F:2 * LOCAL_F],
                        rhs=xk_sb[:, dc, j0:j0 + w],
                        start=(dc == 0), stop=(dc == 3),
                    )
                if qk_bias:
                    nc.vector.tensor_scalar_add(out=ps[:, :w], in0=ps[:, :w],
                                                scalar1=bk_sb)
                # split across ACT+DVE so the first-scores chain is short
                nc.scalar.copy(out=kTp[0][0:64, j0:j0 + w],
                               in_=ps[0:64, :w])
                nc.vector.tensor_copy(out=kTp[1][64:128, j0:j0 + w],
                                      in_=ps[64:128, :w])

            def proj_q(jb, on_act):
                j0 = jb * 512
                ps = ppj.tile([128, 512], F32, tag="pj", name=f"pq{jb}")
                for dc in range(4):
                    nc.tensor.matmul(
                        ps,
                        lhsT=wqk_sb[:, dc, 0:LOCAL_F],
                        rhs=xq_sb[:, dc, j0:j0 + 512],
                        start=(dc == 0), stop=(dc == 3),
                    )
                if qk_bias:
                    nc.vector.tensor_scalar_add(out=ps, in0=ps,
                                                scalar1=bq_sb)
                if on_act:
                    nc.scalar.copy(out=qT[:, j0:j0 + 512], in_=ps)
                else:
                    nc.vector.tensor_copy(out=qT[:, j0:j0 + 512], in_=ps)

            def proj_v(c):
                ps = ppj.tile([128, 512], F32, tag="pj", name=f"pv{c}")
                for dc in range(4):
                    nc.tensor.matmul(
                        ps[:, 0:VAUG],
                        lhsT=xv_sb[:, dc, c * 128:(c + 1) * 128],
                        rhs=wv_sb[:, dc, :],
                        start=(dc == 0), stop=(dc == 3),
                    )
                nc.vector.tensor_copy(out=vaug[:, c, 0:64],
                                      in_=ps[:, 0:64])
                nc.vector.tensor_copy(out=vaug[:, c, 65:129],
                                      in_=ps[:, 65:129])
                if v_bias:
                    nc.vector.tensor_add(out=vaug[:, c, 0:64],
                                         in0=vaug[:, c, 0:64],
                                         in1=bv_sb[:, 0:64])
                    nc.vector.tensor_add(out=vaug[:, c, 65:129],
                                         in0=vaug[:, c, 65:129],
                                         in1=bv_sb[:, 65:129])

            if sk > 512:
                proj_k(0, 512)
                for j0 in range(512, sk, 512):
                    proj_k(j0, min(512, sk - j0))
            else:
                proj_k(0, sk)
            proj_q(0, True)   # ACT evac, in parallel with qp1 on DVE
            proj_q(1, False)
            # priority 0: the list scheduler must not slot filler matmuls
            # (v-proj, later q blocks) ahead of the exp-critical scores
            with tc.high_priority():
                for c in range(PRE):
                    scores(0, c, 0)
                    scores(0, c, 1)
            for v_c in range(nch):
                proj_v(v_c)
            for jb in range(2, seq // 512):
                proj_q(jb, False)

        # ================= attention chunk loops =================
        pool_o = ctx.enter_context(
            tc.tile_pool(name="ps_o", bufs=2, space="PSUM"))
        outp = ctx.enter_context(tc.tile_pool(name="outp", bufs=4))

        def dn_extract(tp, oT, t=None, dn_act=False):
            # pull the denominator rows (psum row 64) into the dn staging
            # tile; t=None copies both 512-halves in one op per head
            dnt = dn[tp % 2]
            sl = slice(0, 1024) if t is None else slice(t * 512,
                                                        (t + 1) * 512)
            for h in range(2):
                dst = dnt[32 * h:32 * h + 1, sl]
                src = oT[h][64:65, sl]
                if dn_act and h == 0:
                    nc.scalar.copy(out=dst, in_=src)
                else:
                    nc.vector.tensor_copy(out=dst, in_=src)

        def norm_bounce_start(tp, oT):
            # boundary normalize, part 1 (emitted in tp's drain): extract
            # f32 denominator rows, bounce through DRAM for the partition
            # broadcast (split over queues), reciprocal on the result, and
            # evacuate the numerators to SBUF to free the psum ring
            for h in range(2):
                nc.vector.tensor_copy(out=dnf[32 * h:32 * h + 1, :],
                                      in_=oT[h][64:65, 0:1024])
                nc.gpsimd.dma_start(out=dnb_d[tp, h, :],
                                    in_=dnf[32 * h:32 * h + 1, :])
                nc.vector.tensor_copy(out=oS[h * 64:(h + 1) * 64, :],
                                      in_=oT[h][0:64, 0:1024])
            for h in range(2):
                bc = dnb_d[tp, h:h + 1, :].to_broadcast([32, 1024])
                nc.gpsimd.dma_start(out=rbf[64 * h:64 * h + 32, :], in_=bc)
                nc.sync.dma_start(out=rbf[64 * h + 32:64 * h + 64, :],
                                  in_=bc)
            nc.vector.reciprocal_approx_fast(out=rbf, in_=rbf)

        def norm_bounce_finish(tp):
            q0 = tp * 1024
            for t in range(2):
                for h in range(2):
                    nc.vector.tensor_mul(
                        out=cn[h * 64:(h + 1) * 64,
                               q0 + t * 512:q0 + (t + 1) * 512],
                        in0=oS[h * 64:(h + 1) * 64,
                               t * 512:(t + 1) * 512],
                        in1=rbf[h * 64:(h + 1) * 64,
                                t * 512:(t + 1) * 512])

        def norm_apply(tp, t, oT):
            q0 = tp * 1024
            sl = slice(q0 + t * 512, q0 + (t + 1) * 512)
            dnt = dn[tp % 2]
            rps = pool_sc.tile([128, 1024], F32, tag="sc",
                               name=f"rb{tp}_{t}")
            nc.tensor.matmul(rps[:, 0:512],
                             lhsT=sel,
                             rhs=dnt[:, t * 512:(t + 1) * 512],
                             start=True, stop=True)
            nc.vector.reciprocal_approx_fast(out=rb[t], in_=rps[:, 0:512])
            for h in range(2):
                nc.vector.tensor_mul(
                    out=cn[h * 64:(h + 1) * 64, sl],
                    in0=oT[h][0:64, t * 512:(t + 1) * 512],
                    in1=rb[t][h * 64:(h + 1) * 64, :])

        dmaq = [nc.gpsimd, nc.sync, nc.scalar]
        nq = [0]

        def outproj_block(tp, oc, pool, evac_act, tag="sc"):
            q0 = tp * 1024
            fp = pool.tile([128, 1024], F32, tag=tag, name=f"fp{tp}_{oc}")
            for t in range(2):
                nc.tensor.matmul(
                    fp[:, t * 512:(t + 1) * 512],
                    lhsT=wo_sb[:, oc * 128:(oc + 1) * 128],
                    rhs=cn[:, q0 + t * 512:q0 + (t + 1) * 512],
                    start=True, stop=True)
            ob = outp.tile([128, 1024], DT, tag="ob", name=f"ob{tp}_{oc}")
            if evac_act and oc % 2 == 0:
                nc.scalar.copy(out=ob, in_=fp)
            else:
                nc.vector.tensor_copy(out=ob, in_=fp)
            dmaq[nq[0] % 2 if not evac_act else nq[0] % 3].dma_start(
                out=out_d[oc * 128:(oc + 1) * 128, q0:q0 + 1024],
                in_=ob)
            nq[0] += 1

        oTs = {}
        for tp in range(n_tp):
            lag = PRE if tp == 0 else min(2, nch)
            prev = tp - 1
            if tp == 0:
                oT = [pool_o.tile([128, 1024], F32, tag="po",
                                  name=f"oT{tp}_{h}") for h in range(2)]
                oTs[tp] = oT
                for c in range(lag, nch):
                    scores(tp, c, 0)
                    scores(tp, c, 1)
                    for h in range(2):
                        attnv(tp, c - lag, h, oT)
            else:
                # pre-emit two chunks of scores so the exp stream crosses
                # the boundary without a bubble; the previous tp's
                # normalize finishes here (bounce already in flight), its
                # out-proj runs on sc-ring slots AFTER the last scores of
                # this tp so no exp-critical matmul ever waits
                for c in range(lag):
                    scores(tp, c, 0)
                    scores(tp, c, 1)
                norm_bounce_finish(prev)
                oT = [pool_o.tile([128, 1024], F32, tag="po",
                                  name=f"oT{tp}_{h}") for h in range(2)]
                oTs[tp] = oT
                for c in range(lag, nch):
                    scores(tp, c, 0)
                    scores(tp, c, 1)
                    for h in range(2):
                        attnv(tp, c - lag, h, oT)
            # drain the attn@v lag; extract denominators as soon as the
            # accumulators stop; the previous tp's out-proj blocks slot
            # into the sc ring behind the last scores
            if tp < n_tp - 1:
                for c in range(max(nch - lag, 0), nch):
                    for h in range(2):
                        attnv(tp, c, h, oT)
                norm_bounce_start(tp, oT)
                if prev >= 0:
                    for oc in range(4):
                        outproj_block(prev, oc, pool_sc, False)
            else:
                for c in range(max(nch - lag, 0), nch - 1):
                    for h in range(2):
                        attnv(tp, c, h, oT)
                attnv(tp, nch - 1, 0, oT, t=0)
                attnv(tp, nch - 1, 1, oT, t=0)
                dn_extract(tp, oT, t=0, dn_act=True)
                attnv(tp, nch - 1, 0, oT, t=1)
                attnv(tp, nch - 1, 1, oT, t=1)
                dn_extract(tp, oT, t=1, dn_act=True)
                # normalize first (it gates the final out-proj), then the
                # previous tp's out-proj blocks on the freed sc-ring slots
                norm_apply(tp, 0, oT)
                norm_apply(tp, 1, oT)
                if prev >= 0:
                    for oc in range(4):
                        outproj_block(prev, oc, pool_sc, True)

        # ================= final out-proj tail (sc ring) =================
        last = n_tp - 1
        for oc in range(4):
            outproj_block(last, oc, pool_sc, True)

    nc.compile()
    return nc


def kernel(queries, keys, values, valid_lens, Wq, bq, Wk, bk, Wv, bv, Wo, bo):
    global last_results
    queries = np.asarray(queries, dtype=np.float32)
    keys = np.asarray(keys, dtype=np.float32)
    values = np.asarray(values, dtype=np.float32)
    valid_lens = np.asarray(valid_lens).astype(np.int64)
    Wq = np.asarray(Wq, dtype=np.float32)
    Wk = np.asarray(Wk, dtype=np.float32)
    Wv = np.asarray(Wv, dtype=np.float32)
    Wo = np.asarray(Wo, dtype=np.float32)
    bq = np.asarray(bq, dtype=np.float32)
    bk = np.asarray(bk, dtype=np.float32)
    bv = np.asarray(bv, dtype=np.float32)
    bo = np.asarray(bo, dtype=np.float32)

    B, S, D = queries.shape
    assert (B, D) == (2, D_MODEL) and S % 1024 == 0

    Lmax = int(min(max(int(valid_lens.max()), 1), S))
    nch = (Lmax + 127) // 128
    sk = nch * 128

    npdt = _np_dt(DT_NAME)
    np8 = ml_dtypes.float8_e4m3
    qk_bias = bool(np.any(bq) or np.any(bk))
    v_bias = bool(np.any(bv))
    key = (nch, S, DT_NAME, qk_bias, v_bias)
    if key not in _PROG_CACHE:
        _PROG_CACHE[key] = _build(nch, S, DT_NAME, qk_bias, v_bias)
    nc = _PROG_CACHE[key]

    def pmaj(x, dt):
        # [D, f] feature-major -> [128, 4, f] partition-major
        f = x.shape[1]
        return np.ascontiguousarray(
            x.reshape(4, 128, f).transpose(1, 0, 2)).astype(dt)

    hs = S // 2
    in_maps = []
    for core in range(N_CORES):
        b, hp = divmod(core, 4)
        L = int(valid_lens[b])
        fs = hp * LOCAL_F
        wvT_aug = np.zeros((D, VAUG), np.float32)
        wvT_aug[:, 0:64] = Wv[fs:fs + 64, :].T
        wvT_aug[:, 65:129] = Wv[fs + 64:fs + 128, :].T
        bv_aug = np.zeros((VAUG,), np.float32)
        bv_aug[0:64] = bv[fs:fs + 64]
        bv_aug[64] = 1.0
        bv_aug[65:129] = bv[fs + 64:fs + 128]
        bv_aug[129] = 1.0
        if L == 0:
            mask = np.zeros((sk,), np.float32)  # result discarded on host
        else:
            mask = np.where(np.arange(sk) < L, 0.0, MASK_NEG).astype(np.float32)
        wqk = np.concatenate(
            [Wq[fs:fs + 128, :].T, Wk[fs:fs + 128, :].T], axis=1) * WSCALE
        smalls = np.empty((128, 2 + VAUG + nch), np.float32)
        smalls[:, 0] = bq[fs:fs + 128] * WSCALE
        smalls[:, 1] = bk[fs:fs + 128] * WSCALE
        smalls[:, 2:2 + VAUG] = bv_aug
        smalls[:, 2 + VAUG:] = mask.reshape(nch, 128).T
        sel = np.zeros((128, 128), np.float32)
        sel[0, 0:64] = 1.0
        sel[32, 64:128] = 1.0
        xq = np.ascontiguousarray(queries[b].T)
        im = {
            "sel": sel.astype(npdt),
            "xk": pmaj(np.ascontiguousarray(keys[b, :sk].T), np8),
            "xv": pmaj(np.ascontiguousarray(values[b, :sk].T), npdt),
            "wqk": pmaj(wqk, np8),
            "wv": pmaj(wvT_aug, npdt),
            "wo": np.ascontiguousarray(Wo[:, fs:fs + 128].T).astype(npdt),
            "smalls": smalls,
        }
        for i in range(S // 512):
            im[f"xq{i}"] = pmaj(xq[:, i * 512:(i + 1) * 512], np8)
        in_maps.append(im)

    from concourse.bass_utils import run_bass_kernel_spmd
    res = run_bass_kernel_spmd(nc, in_maps, list(range(N_CORES)), trace=TRACE)
    last_results = res
    outs = [r["out"] for r in res.results]

    final = np.empty((B, S, D), np.float32)
    for b in range(B):
        acc = sum(outs[4 * b + i].astype(np.float32) for i in range(4))
        final[b] = acc.T + bo
        if int(valid_lens[b]) == 0:
            # uniform attention over all S positions (reference semantics
            # when every key is masked: softmax of a constant row)
            row = (values[b].mean(0) @ Wv.T + bv) @ Wo.T + bo
            final[b] = np.broadcast_to(row, (S, D))
    return final
